# revision 16
# baseline (speedup 1.0000x reference)
"""Grouped-query attention (B=8,S=512,D=4096,G=32) on 8 trn2 cores.

Data-parallel over batch: core b handles batch b. Per core everything is
computed in a feature-major layout (no on-device transposes):

  qT[f,t] accumulates 32 matmuls of wq-tile.T @ x-tile per 128-feature
  block (= per head); same for kT; v token-major. RoPE on heads 0..7
  (per-token scalar angle; head g pairs with g+4), processed LAST so the
  first matmuls don't wait on the trig setup. ALPHA is folded into xq on
  the host. Attention per head: sT = k_blk.T @ qT, exp via ACT (mask as
  bias), o accumulated from v blocks; softmax denominators accumulate
  per 16-head group into [16,512] PSUM tiles via one-hot matmuls ->
  one reciprocal per group -> per-head rank-1 broadcast matmul ->
  in-place normalize. Group 0 normalizes while group 1's heads compute;
  group 1 normalizes while the output projection starts. Unnormalized o
  overwrites v's SBUF space (head g's v is dead once its o-matmuls
  issued). Output projection token-major.

DMA: two HWDGE rings. ACT ring (nc.scalar): xq, trig/mask, xk, ssel/
bsel, xv, wv quarter-slabs. SP ring (nc.sync): wq, wk, wo, y. All DRAM
parameters are host-pre-tiled so every transfer is contiguous.
"""

import math

import numpy as np
import ml_dtypes

import concourse.bass as bass
import concourse.mybir as mybir
import concourse.tile as tile
from concourse import bacc
from concourse.bass_utils import run_bass_kernel_spmd

B, S, D = 8, 512, 4096
G, DH = 32, 128
RD = 1024
ALPHA = 1.0 / math.sqrt(DH)
PI = math.pi
NCORES = 8
GH = 16  # softmax normalization group size (heads)
DT = mybir.dt
AF = mybir.ActivationFunctionType
ALU = mybir.AluOpType

# set by test.py to capture a profile
TRACE = False
LAST_RESULT = None


def _range_reduce(nc, ang, mtmp):
    """In-place reduce ang (>=0, < ~8*pi) into (-pi, pi] mod 2*pi."""
    for _ in range(4):
        nc.vector.tensor_scalar(mtmp, ang, PI, 2.0 * PI, ALU.is_gt, ALU.mult)
        nc.vector.tensor_sub(ang, ang, mtmp)


def build_program():
    nc = bacc.Bacc(
        "TRN2", target_bir_lowering=False, debug=False, num_devices=NCORES
    )
    bf16 = DT.bfloat16
    f32 = DT.float32

    xq_d = nc.declare_dram_parameter("xq", [128, 32, S], bf16, isOutput=False)
    xk_d = nc.declare_dram_parameter("xk", [128, 32, S], bf16, isOutput=False)
    xv_d = nc.declare_dram_parameter("xv", [128, 32, S], bf16, isOutput=False)
    wq_d = nc.declare_dram_parameter(
        "wq", [32, 128, 32, 128], bf16, isOutput=False
    )
    wk_d = nc.declare_dram_parameter(
        "wk", [32, 128, 32, 128], bf16, isOutput=False
    )
    wv_d = nc.declare_dram_parameter(
        "wv", [8, 128, 32, 512], bf16, isOutput=False
    )
    wo_d = nc.declare_dram_parameter(
        "wo", [8, 128, 32, 512], bf16, isOutput=False
    )
    pos_d = nc.declare_dram_parameter("pos", [S], f32, isOutput=False)
    invf_d = nc.declare_dram_parameter("invf", [S], f32, isOutput=False)
    mask_d = nc.declare_dram_parameter("maskin", [S], DT.int32, isOutput=False)
    ssel_d = nc.declare_dram_parameter(
        "ssel", [128, GH * GH], bf16, isOutput=False
    )
    bsel_d = nc.declare_dram_parameter(
        "bsel", [GH, GH * 128], bf16, isOutput=False
    )
    y_d = nc.declare_dram_parameter("y", [32, 128, 512], f32, isOutput=True)

    with tile.TileContext(nc) as tc:
        with tc.tile_pool(name="persist", bufs=1) as persist:
            sin_t = persist.tile([128, S], bf16, tag="sin")
            cos_t = persist.tile([128, S], bf16, tag="cos")
            maskb = persist.tile([128, 4], f32, tag="maskb")
            ones_f1 = persist.tile([1, 128], f32, tag="ones_f1")
            ssel_s = persist.tile([128, GH * GH], bf16, tag="ssel")
            bsel_s = persist.tile([GH, GH * 128], bf16, tag="bsel")
            qT_s = persist.tile([128, G, S], bf16, tag="qT")
            kT_s = persist.tile([128, G, S], bf16, tag="kT")
            # v during attention; overwritten per head by unnormalized oT
            # (stored so that v_s[:, tb, g, :] == attnT[g][:, tb*128:+128])
            v_s = persist.tile([128, 4, G, DH], bf16, tag="v")

            nc.vector.memset(ones_f1, 1.0)

            # RoPE head-pairs sit mid-stage: the first matmuls (plain
            # heads) depend only on xq + the first wq slab, and the last
            # chains release their PSUM banks via fast ACT copies so the
            # next stage's matmuls aren't gated on slow DVE tails.
            fb_order = (
                list(range(8, 20))
                + [0, 4, 1, 5, 2, 6, 3, 7]
                + list(range(20, 32))
            )

            with (
                tc.tile_pool(name="xin", bufs=2) as xin_pool,
                tc.tile_pool(name="wsl", bufs=3) as w_pool,
                tc.tile_pool(name="psqk", bufs=6, space="PSUM") as psqk_pool,
            ):
                xq_s = xin_pool.tile([128, 32, S], bf16, tag="x")
                for qx in range(4):
                    nc.scalar.dma_start(
                        out=xq_s[:, qx * 8 : (qx + 1) * 8, :],
                        in_=xq_d[:, qx * 8 : (qx + 1) * 8, :],
                    )
                # first weight half-slabs: allocated before the setup pool
                # opens so their SBUF slots never overlap setup tiles (an
                # overlap makes the first weight DMA wait ~30us for the
                # trig chain to release its space)
                w_first = w_pool.tile([128, 32, 128], bf16, tag="w")
                nc.sync.dma_start(out=w_first, in_=wq_d[fb_order[0], :, :, :])

                # ---- setup: trig + mask ----
                with (
                    tc.tile_pool(name="setup", bufs=1) as setup,
                    tc.tile_pool(name="ps_setup", bufs=2, space="PSUM") as ps_setup,
                ):
                    pos1 = setup.tile([1, S], f32, tag="pos1")
                    invf1 = setup.tile([1, S], f32, tag="tmp")
                    angc = setup.tile([1, S], f32, tag="angc")
                    mi = setup.tile([128, 4], DT.int32, tag="mi")
                    mf = setup.tile([128, 4], f32, tag="mf")

                    nc.scalar.dma_start(out=pos1, in_=pos_d[None, :])
                    nc.scalar.dma_start(out=invf1, in_=invf_d[None, :])
                    angs = pos1
                    nc.vector.tensor_mul(angs, pos1, invf1)
                    mtmp = setup.tile([1, S], f32, tag="tmp")
                    nc.vector.tensor_scalar_add(angc, angs, PI / 2.0)
                    _range_reduce(nc, angs, mtmp)
                    _range_reduce(nc, angc, mtmp)
                    nc.scalar.activation(angs, angs, AF.Sin)
                    nc.scalar.activation(angc, angc, AF.Sin)
                    ps_sin = ps_setup.tile([128, S], f32, tag="b", name="ps_sin")
                    ps_cos = ps_setup.tile([128, S], f32, tag="b", name="ps_cos")
                    nc.tensor.matmul(ps_sin, ones_f1, angs, start=True, stop=True)
                    nc.tensor.matmul(ps_cos, ones_f1, angc, start=True, stop=True)
                    nc.scalar.copy(sin_t, ps_sin)
                    nc.scalar.copy(cos_t, ps_cos)

                    nc.scalar.dma_start(
                        out=mi, in_=mask_d[:].rearrange("(b p) -> p b", p=128)
                    )
                    nc.vector.tensor_copy(mf, mi)
                    nc.vector.tensor_scalar(
                        maskb, mf, 1.0e9, 1.0e9, ALU.mult, ALU.subtract
                    )
                    # preload the Exp ACT table set during the projections
                    # so attention's first exp doesn't stall on a table load
                    nc.scalar.activation(mtmp, mtmp, AF.Exp, scale=0.0)

                xk_s = xin_pool.tile([128, 32, S], bf16, tag="x")
                for qx in range(4):
                    nc.scalar.dma_start(
                        out=xk_s[:, qx * 8 : (qx + 1) * 8, :],
                        in_=xk_d[:, qx * 8 : (qx + 1) * 8, :],
                    )

                # ---- q and k projections (feature-major out) + RoPE ----
                for which, w_d, x_s, outT in (
                    ("q", wq_d, xq_s, qT_s),
                    ("k", wk_d, xk_s, kT_s),
                ):
                    with tc.tile_pool(name=f"rt{which}", bufs=2) as rtmp_pool:
                        pair_ps = {}
                        for fi, fb in enumerate(fb_order):
                            if which == "q" and fi == 0:
                                wt = w_first
                            else:
                                wt = w_pool.tile(
                                    [128, 32, 128], bf16, tag="w"
                                )
                                nc.sync.dma_start(
                                    out=wt, in_=w_d[fb, :, :, :]
                                )
                            ps = psqk_pool.tile(
                                [128, S], f32, tag="ps", name="ps_qk"
                            )
                            for db in range(32):
                                nc.tensor.matmul(
                                    ps,
                                    wt[:, db, :],
                                    x_s[:, db, :],
                                    start=(db == 0),
                                    stop=(db == 31),
                                )
                            if fb >= 8:
                                nc.scalar.copy(outT[:, fb, :], ps)
                            elif fb < 4:
                                pair_ps[fb] = ps
                            else:
                                g = fb - 4
                                ps_a, ps_b = pair_ps.pop(g), ps
                                t1 = rtmp_pool.tile([128, S], bf16, tag="t")
                                t2 = rtmp_pool.tile([128, S], bf16, tag="t")
                                nc.vector.tensor_mul(t1, ps_a, cos_t)
                                nc.vector.tensor_mul(t2, ps_b, sin_t)
                                nc.vector.tensor_sub(outT[:, g, :], t1, t2)
                                t3 = rtmp_pool.tile([128, S], bf16, tag="t")
                                t4 = rtmp_pool.tile([128, S], bf16, tag="t")
                                nc.vector.tensor_mul(t3, ps_b, cos_t)
                                nc.vector.tensor_mul(t4, ps_a, sin_t)
                                nc.vector.tensor_add(outT[:, g + 4, :], t3, t4)
                    if which == "q":
                        # prefetch xv into the slot xq_s releases
                        xv_s = xin_pool.tile([128, 32, S], bf16, tag="x")
                        for qx in range(4):
                            nc.scalar.dma_start(
                                out=xv_s[:, qx * 8 : (qx + 1) * 8, :],
                                in_=xv_d[:, qx * 8 : (qx + 1) * 8, :],
                            )

                # attention's selector constants: issued here so they land
                # on the ACT ring ahead of the wv slabs
                nc.scalar.dma_start(out=ssel_s, in_=ssel_d[:, :])
                nc.scalar.dma_start(out=bsel_s, in_=bsel_d[:, :])

                # ---- v projection (token-major) ----
                # Weight eighth-slabs (same 4KiB slot size as the q/k
                # halves) are walked sequentially inside the accumulation
                # chain so each releases after its 16 matmuls.
                if True:
                    for fc in range(8):
                        pss = [
                            psqk_pool.tile([128, 512], f32, tag="ps", name="ps_v")
                            for _ in range(4)
                        ]
                        for qi in range(4):
                            wt = w_pool.tile([128, 8, 512], bf16, tag="w")
                            nc.scalar.dma_start(
                                out=wt, in_=wv_d[fc, :, qi * 8 : (qi + 1) * 8, :]
                            )
                            for dl in range(8):
                                db = qi * 8 + dl
                                for tb in range(4):
                                    nc.tensor.matmul(
                                        pss[tb],
                                        xv_s[:, db, tb * 128 : (tb + 1) * 128],
                                        wt[:, dl, :],
                                        start=(db == 0),
                                        stop=(db == 31),
                                    )
                        for tb in range(4):
                            nc.vector.tensor_copy(
                                v_s[:, tb, fc * 4 : (fc + 1) * 4, :], pss[tb]
                            )

            # ---- attention ----
            # Softmax denominators accumulate per 16-head group; broadcast
            # tiles reuse the finished group's sums slot, so PSUM is
            # 2 score + 4 o + 2 sums/rb = 8 banks.
            with (
                tc.tile_pool(name="rsm", bufs=1) as rpool,
                tc.tile_pool(name="ps_m", bufs=1, space="PSUM") as ps_m_pool,
            ):
                sums_t = [
                    ps_m_pool.tile([GH, S], f32, tag=f"sums{i}", name=f"sums{i}")
                    for i in range(2)
                ]

                def normalize_group(grp):
                    r_g = rpool.tile([GH, S], bf16, tag=f"r{grp}")
                    with nc.allow_low_precision(
                        reason="softmax reciprocal in bf16 is within tolerance"
                    ):
                        nc.vector.reciprocal(r_g, sums_t[grp])
                    for g in range(grp * GH, (grp + 1) * GH):
                        ps_rb = ps_m_pool.tile(
                            [128, S], f32, tag=f"sums{grp}", name="ps_rb"
                        )
                        nc.tensor.matmul(
                            ps_rb,
                            bsel_s[:, (g % GH) * 128 : (g % GH + 1) * 128],
                            r_g,
                            start=True,
                            stop=True,
                        )
                        nc.vector.tensor_mul(
                            v_s[:, :, g, :],
                            v_s[:, :, g, :],
                            ps_rb[:, :].rearrange("p (a b) -> p a b", a=4),
                        )

                with (
                    tc.tile_pool(name="wexp", bufs=8) as wexp_pool,
                    tc.tile_pool(name="ps_s", bufs=3, space="PSUM") as ps_s_pool,
                    tc.tile_pool(name="ps_o", bufs=3, space="PSUM") as ps_o_pool,
                ):
                    for g in range(G):
                        grp, gl = divmod(g, GH)
                        ps_o = ps_o_pool.tile([128, S], f32, tag="o")
                        for kb in range(4):
                            ps_sc = ps_s_pool.tile([128, S], f32, tag="s")
                            nc.tensor.matmul(
                                ps_sc,
                                kT_s[:, g, kb * 128 : (kb + 1) * 128],
                                qT_s[:, g, :],
                                start=True,
                                stop=True,
                            )
                            wb = wexp_pool.tile([128, S], bf16, tag="w")
                            nc.scalar.activation(
                                wb,
                                ps_sc,
                                AF.Exp,
                                bias=maskb[:, kb : kb + 1],
                                scale=1.0,
                            )
                            nc.tensor.matmul(
                                ps_o,
                                v_s[:, kb, g, :],
                                wb,
                                start=(kb == 0),
                                stop=(kb == 3),
                            )
                            nc.tensor.matmul(
                                sums_t[grp],
                                ssel_s[:, gl * GH : (gl + 1) * GH],
                                wb,
                                start=(gl == 0 and kb == 0),
                                stop=(gl == GH - 1 and kb == 3),
                            )
                        # unnormalized oT -> v_s space of head g (v now
                        # dead): v_s[:, tb, g, :] <- ps_o[:, tb*128:+128]
                        nc.vector.tensor_copy(
                            v_s[:, :, g, :],
                            ps_o[:, :].rearrange("p (a b) -> p a b", a=4),
                        )
                        if g == GH - 1:
                            normalize_group(0)
                # score/o banks are free here; the final normalize overlaps
                # the output projection's first chains below.
                normalize_group(1)

                # ---- y = attn @ Wo.T (token-major out) ----
                with (
                    tc.tile_pool(name="wo", bufs=4) as w_pool,
                    tc.tile_pool(name="psy", bufs=6, space="PSUM") as ps_pool,
                    tc.tile_pool(name="yout", bufs=4) as y_pool,
                ):
                    for fc in range(8):
                        pss = [
                            ps_pool.tile([128, 512], f32, tag="ps", name="ps_y")
                            for _ in range(4)
                        ]
                        for qi in range(4):
                            wt = w_pool.tile([128, 8, 512], bf16, tag="w")
                            nc.sync.dma_start(
                                out=wt, in_=wo_d[fc, :, qi * 8 : (qi + 1) * 8, :]
                            )
                            for dl in range(8):
                                db = qi * 8 + dl
                                for tb in range(4):
                                    nc.tensor.matmul(
                                        pss[tb],
                                        v_s[:, tb, db, :],
                                        wt[:, dl, :],
                                        start=(db == 0),
                                        stop=(db == 31),
                                    )
                        for tb in range(4):
                            yt = y_pool.tile([128, 512], f32, tag="y")
                            nc.vector.tensor_copy(yt, pss[tb])
                            nc.sync.dma_start(
                                out=y_d[fc * 4 + tb, :, :], in_=yt
                            )
    nc.compile()
    return nc


_NC_CACHE = None


def _get_program():
    global _NC_CACHE
    if _NC_CACHE is None:
        _NC_CACHE = build_program()
    return _NC_CACHE


def make_in_maps(query, key, value, mask, position_ids, Wq, Wk, Wv, Wo):
    bf16 = ml_dtypes.bfloat16

    def qk_tile(W):  # [4096,4096] -> [32 fb, 128 p, 32 db, 128 f]
        t = np.asarray(W, np.float32).astype(bf16)
        t = t.reshape(32, 128, 32, 128)  # [fb, f, db, p]
        return np.ascontiguousarray(t.transpose(0, 3, 2, 1))

    def vo_tile(W):  # [4096,4096] -> [8 fc, 128 p, 32 db, 512 f]
        t = np.asarray(W, np.float32).astype(bf16)
        t = t.reshape(8, 512, 32, 128)  # [fc, f, db, p]
        return np.ascontiguousarray(t.transpose(0, 3, 2, 1))

    def x_tile(x, scale=None):  # [512,4096] -> [128 p, 32 db, 512 t]
        x = np.asarray(x, np.float32)
        if scale is not None:
            x = x * scale
        t = x.astype(bf16).T.reshape(32, 128, S)  # [db, p, t]
        return np.ascontiguousarray(t.transpose(1, 0, 2))

    wq = qk_tile(np.asarray(Wq))
    wk = qk_tile(np.asarray(Wk))
    wv = vo_tile(np.asarray(Wv))
    wo = vo_tile(np.asarray(Wo))
    invf = (10000.0 ** (-np.arange(0, RD, 2, dtype=np.float32) / RD)).astype(
        np.float32
    )
    ssel = np.zeros((128, GH * GH), bf16)
    for a in range(GH):
        ssel[:, a * GH + a] = 1
    bsel = np.zeros((GH, GH * 128), bf16)
    for a in range(GH):
        bsel[a, a * 128 : (a + 1) * 128] = 1

    in_maps = []
    for b in range(NCORES):
        in_maps.append(
            {
                "xq": x_tile(query[b], ALPHA),
                "xk": x_tile(key[b]),
                "xv": x_tile(value[b]),
                "wq": wq,
                "wk": wk,
                "wv": wv,
                "wo": wo,
                "pos": np.ascontiguousarray(
                    np.asarray(position_ids[b], np.float32)
                ),
                "invf": invf,
                "maskin": np.ascontiguousarray(np.asarray(mask[b], np.int32)),
                "ssel": ssel,
                "bsel": bsel,
            }
        )
    return in_maps


def unshard_y(y_tiles):
    # [32, 128, 512] blocks (fc*4+tb) -> [512, 4096]
    return (
        y_tiles.reshape(8, 4, 128, 512)
        .transpose(1, 2, 0, 3)
        .reshape(S, D)
    )


def kernel(query, key, value, mask, position_ids, Wq, Wk, Wv, Wo):
    global LAST_RESULT
    nc = _get_program()
    in_maps = make_in_maps(
        query, key, value, mask, position_ids, Wq, Wk, Wv, Wo
    )
    res = run_bass_kernel_spmd(
        nc, in_maps, core_ids=list(range(NCORES)), trace=TRACE
    )
    LAST_RESULT = res
    out = np.stack(
        [unshard_y(np.asarray(res.results[b]["y"])) for b in range(NCORES)],
        axis=0,
    )
    return np.ascontiguousarray(out.astype(np.float32))
